# revision 6
# baseline (speedup 1.0000x reference)
# Self-contained Trainium2 Bass kernel for:
#   scores = Q @ K.T            [N, M]
#   attn   = softmax(scores, axis=0)   (over queries, per key column)
#   out    = attn @ V           [N, D]
# with N = M = 8192, D = 128, float32 I/O.
#
# Sharding: K/V rows (the M axis) are split across the 8 NeuronCores.
# The softmax axis (N) stays fully local to each core, so no collectives
# are needed: each core produces a partial out^T = sum over its M-shard,
# and the host sums the 8 partials.
#
# Device algorithm (per core, M_SH = 1024):
#   scoresT = K_sh @ Q^T        [M_SH, N]   (PE, fp16 inputs, f32 PSUM)
#   expT    = exp(scoresT)      bf16, via ScalarE directly from PSUM,
#                               with fused accum_out row-sums -> denom[m]
#   V'      = V / denom[:,None] bf16 (fold softmax normalizer into V)
#   outT    = V'^T @ expT       [D, N]      (PE, bf16, accumulated in PSUM)
#
# No max-subtraction is needed: scores ~ N(0, 128), |s| < ~70, and
# exp(70) ~ 2.5e30 fits fp32/bf16 range comfortably.
#
# Layouts: the contraction dim of phase 1 is D=128, which must sit on the
# SBUF partition axis for the PE; the host passes Q^T and K_sh^T so every
# DMA is a contiguous load and the device never transposes anything.

from contextlib import ExitStack

import numpy as np

import concourse.bass as bass
import concourse.mybir as mybir
import concourse.tile as tile
from concourse import bacc
from concourse.bass_utils import run_bass_kernel_spmd

N, M, D = 8192, 8192, 128
N_CORES = 8
M_SH = M // N_CORES  # 1024

F32 = mybir.dt.float32
F16 = mybir.dt.float16
BF16 = mybir.dt.bfloat16


def build_attention_nc(
    n=N, m_sh=M_SH, d=D, mm_chunk=512, exp_chunk=2048, reps=1,
    timer_k=0, timer_p=2400, layout="serial", group_mts=2, psum_bufs=None,
    rowsum="act", p2_chunk=None, groups=None, p2_own_slots=False,
    outp_bufs=3, out_f16=False, dve_per_group=None,
):
    """Build the per-core Bass program.

    mm_chunk: free-dim (n) size of each phase-1/phase-2 matmul (<=512, one
              f32 PSUM bank per matmul).
    exp_chunk: free-dim size of each ScalarE exp op; one PSUM tile of
               exp_chunk/mm_chunk banks is filled by that many matmuls and
               consumed by a single activation instruction.
    timer_k: if >0, add an on-device timing sampler: the (otherwise idle)
             GpSimd engine snapshots two SBUF flag cells every ~timer_p
             cycles into a [2, timer_k] "tsamp" output.  Flag A is written
             (via +1e30) when phase 1 finishes, flag B when the last
             phase-2 copy finishes.  The flag cells are aliased SBUF
             tensors (alloc_sbuf_tensor_at) so the sampler's reads are
             invisible to Tile's dependency tracker and genuinely race
             with the compute - which is the whole point.  Timing-only
             variant; the graded kernel() path uses timer_k=0.
    """
    assert d == 128
    assert m_sh % 128 == 0 and n % exp_chunk == 0 and exp_chunk % mm_chunk == 0
    MT = m_sh // 128           # m-tiles of 128 partitions
    ECH = n // exp_chunk       # exp chunks per m-tile
    MM_PER_E = exp_chunk // mm_chunk
    NCH = n // mm_chunk        # phase-2 output chunks

    nc = bacc.Bacc()
    qt = nc.dram_tensor("qt", [d, n], F16, kind="ExternalInput")
    kt = nc.dram_tensor("kt", [d, m_sh], F16, kind="ExternalInput")
    v = nc.dram_tensor("v", [m_sh, d], F32, kind="ExternalInput")
    ot = nc.dram_tensor("ot", [d, n], F16 if out_f16 else F32, kind="ExternalOutput")

    flags_w = flags_r = tsamp = None
    U32 = mybir.dt.int32
    if timer_k:
        tsamp = nc.dram_tensor("tsamp", [2, timer_k], U32, kind="ExternalOutput")
        # Two views of the same SBUF bytes: the compute side writes flags_w
        # (f32), the sampler reads the raw bits through flags_r (i32;
        # reg_load/save require int dtypes).  flags_w is bump-allocated
        # normally (so nothing else lands there) and flags_r aliases its
        # offset via alloc_sbuf_tensor_at.
        flags_w = nc.alloc_sbuf_tensor("flags_w", [1, 2], F32).ap()
        flag_addr = next(
            a.memorylocations[0].addr
            for a in nc.m.functions[0].allocations
            if getattr(a, "memorylocations", None)
            and a.memorylocations[0].name.startswith("flags_w")
        )
        flags_r = nc.alloc_sbuf_tensor_at(
            "flags_r", [1, 2], U32, offset=flag_addr
        ).ap()

    with tile.TileContext(nc) as tc, ExitStack() as ctx:
        singles = ctx.enter_context(tc.tile_pool(name="singles", bufs=1))
        # One PSUM pool; phase-1 exp tiles and phase-2 accumulators share the
        # same tag, together filling all 8 banks.
        if psum_bufs is None:
            psum_bufs = 4 if layout in ("overlap", "overlap2") else 2
        psum = ctx.enter_context(
            tc.tile_pool(name="psum", bufs=psum_bufs, space="PSUM")
        )
        outp = ctx.enter_context(tc.tile_pool(name="outp", bufs=outp_bufs))

        # kt first (small, needed by the very first matmul), then qt in
        # chunks so matmul 0 isn't gated on the full 2 MB load, v last.
        kt_s = singles.tile([d, m_sh], F16)
        nc.sync.dma_start(out=kt_s, in_=kt[:, :])
        qt_s = singles.tile([d, n], F16)
        n_ld = min(max(exp_chunk, n // 8), n // 16)
        for i in range(n // n_ld):
            nc.sync.dma_start(
                out=qt_s[:, i * n_ld : (i + 1) * n_ld],
                in_=qt[:, i * n_ld : (i + 1) * n_ld],
            )
        v_s = singles.tile([128, MT, d], F32)
        nc.sync.dma_start(out=v_s, in_=v.rearrange("(t p) d -> p t d", p=128))
        # Warm the ScalarE exp table during the input-DMA window so the
        # ~2.7us ACT_TABLE_LOAD is off the critical path of the first real
        # exp op.
        actwarm = singles.tile([1, 1], F32, name="actwarm")
        nc.vector.memset(actwarm, 0.0)
        actwarm2 = singles.tile([1, 1], F32, name="actwarm2")
        nc.scalar.activation(
            out=actwarm2, in_=actwarm, func=mybir.ActivationFunctionType.Exp
        )
        # First-touch v_s on DVE: the TS (tensor_scalar) instruction format
        # has a single HW sync-wait slot, so the real consumer below must not
        # be the one that waits on this DMA.
        v_touch = singles.tile([128, 1], F32)
        nc.vector.tensor_copy(v_touch, v_s[:, 0, 0:1])

        expT = [
            singles.tile([128, n], BF16, tag=f"expT{mt}", name=f"expT{mt}")
            for mt in range(MT)
        ]
        dch = [
            singles.tile([128, ECH], F32, tag=f"dch{mt}", name=f"dch{mt}")
            for mt in range(MT)
        ]
        denom = singles.tile([128, MT], F32)
        recip = singles.tile([128, MT], F32)
        vb = singles.tile([128, MT, d], BF16)
        outacc = (
            singles.tile([128, n], F16, name="outacc")
            if layout in ("overlap", "overlap2")
            else None
        )
        # Garbage output for the DVE tensor_scalar that computes the row
        # sums (accum_out) at 4x off the bf16 expT chunks; rewritten every
        # call, same engine so pure program-order, no sync cost.
        tsscr = singles.tile([128, exp_chunk], BF16, name="tsscr")

        if timer_k:
            gp = nc.gpsimd
            gp.memset(flags_r, 0)
            sampA = singles.tile([1, timer_k], U32)
            sampB = singles.tile([1, timer_k], U32)
            # Pacing: a Pool-engine memset of timer_p elements (~timer_p
            # cycles @1.2GHz + Q7 launch overhead).
            pace = singles.tile([1, timer_p], U32)
            rA = gp.alloc_register("rA")
            rB = gp.alloc_register("rB")
            for i in range(timer_k):
                gp.memset(pace, 0)
                gp.reg_load(rA, flags_r[0:1, 0:1])
                gp.reg_save(sampA[0:1, i : i + 1], rA)
                gp.reg_load(rB, flags_r[0:1, 1:2])
                gp.reg_save(sampB[0:1, i : i + 1], rB)
            gp.dma_start(out=tsamp[0:1, :], in_=sampA)
            gp.dma_start(out=tsamp[1:2, :], in_=sampB)

        # reps>1 repeats the whole compute body inside one NEFF; used only by
        # the timing harness (per-dispatch overhead cancels in the delta).
        for _rep in range(reps):
            if layout == "overlap2":
                run_body_overlap2(
                    nc, psum, outp, qt_s, kt_s, v_s, expT, dch, denom, recip,
                    vb, outacc, ot, MT, ECH, MM_PER_E, mm_chunk, exp_chunk,
                    list(groups), list(dve_per_group), flags_w=flags_w,
                    tsscr=tsscr, p2_chunk=p2_chunk,
                )
            elif layout == "overlap":
                run_body_overlap(
                    nc, psum, outp, qt_s, kt_s, v_s, expT, dch, denom, recip,
                    vb, outacc, ot, MT, ECH, MM_PER_E, mm_chunk, exp_chunk,
                    group_mts, flags_w=flags_w, tsscr=tsscr, rowsum=rowsum,
                    p2_chunk=p2_chunk, groups=groups, p2_own_slots=p2_own_slots,
                )
            else:
                run_body(
                    nc, tc, psum, outp, qt_s, kt_s, v_s, expT, dch, denom,
                    recip, vb, ot, MT, ECH, MM_PER_E, NCH, mm_chunk, exp_chunk,
                    flags_w=flags_w, tsscr=tsscr, rowsum=rowsum,
                )

    nc.compile()
    return nc


def _exp_rowsum(nc, tsscr, expT_slice, dch_slice):
    # Row-sum of a bf16 expT chunk on the DVE at 4x (all-SBUF, 2-byte
    # operands; the f32 accum_out scalar is exempt).  ~0.26 ns/elem vs
    # 187 ns of serial ACT time for activation(accum_out=...).
    nc.vector.tensor_scalar(
        out=tsscr[:, : expT_slice.shape[-1]],
        in0=expT_slice,
        scalar1=1.0,
        scalar2=None,
        op0=mybir.AluOpType.mult,
        op1=mybir.AluOpType.add,
        accum_out=dch_slice,
    )


def run_body_overlap(
    nc, psum, outp, qt_s, kt_s, v_s, expT, dch, denom, recip, vb, outacc,
    ot, MT, ECH, MM_PER_E, mm_chunk, exp_chunk, group_mts, flags_w=None,
    tsscr=None, rowsum="act", p2_chunk=None, groups=None, p2_own_slots=False,
):
    """Group the m-tiles; after each group's phase 1, its phase-2 partial
    (outT contribution) is emitted interleaved into the NEXT group's
    phase-1 stream, accumulating into fp16 outacc.  Only the last group's
    phase-2 remains as a serial tail (~1/n_groups of the old 28us)."""
    d = vb.shape[-1]
    n = qt_s.shape[-1]
    if groups is None:
        groups = [group_mts] * (MT // group_mts)
    assert sum(groups) == MT
    n_groups = len(groups)
    starts = [sum(groups[:i]) for i in range(n_groups)]

    def mts_of(g):
        return list(range(starts[g], starts[g] + groups[g]))
    # Interleaved groups use narrow p2 tiles (less slot-hold disruption of
    # the ACT exp feed); the final tail group uses wide ones (fewer drain
    # ops on the critical tail).
    P2C_MID = p2_chunk or exp_chunk
    # With dedicated p2 slots (2 x 1-bank), every p2 tile must fit 1 bank.
    P2C_LAST = P2C_MID if p2_own_slots else exp_chunk

    def emit_exp(mt, e):
        k_col = kt_s[:, mt * 128 : (mt + 1) * 128]
        ps = psum.tile([128, exp_chunk], F32, tag="ps", name="ps")
        for j in range(MM_PER_E):
            c0 = e * exp_chunk + j * mm_chunk
            nc.tensor.matmul(
                ps[:, j * mm_chunk : (j + 1) * mm_chunk],
                lhsT=k_col,
                rhs=qt_s[:, c0 : c0 + mm_chunk],
                start=True,
                stop=True,
            )
        if rowsum == "act":
            nc.scalar.activation(
                out=expT[mt][:, e * exp_chunk : (e + 1) * exp_chunk],
                in_=ps,
                func=mybir.ActivationFunctionType.Exp,
                accum_out=dch[mt][:, e : e + 1],
            )
        else:
            if rowsum == "act":
                nc.scalar.activation(
                    out=expT[mt][:, e * exp_chunk : (e + 1) * exp_chunk],
                    in_=ps,
                    func=mybir.ActivationFunctionType.Exp,
                    accum_out=dch[mt][:, e : e + 1],
                )
            else:
                nc.scalar.activation(
                    out=expT[mt][:, e * exp_chunk : (e + 1) * exp_chunk],
                    in_=ps,
                    func=mybir.ActivationFunctionType.Exp,
                )
                _exp_rowsum(
                    nc, tsscr,
                    expT[mt][:, e * exp_chunk : (e + 1) * exp_chunk],
                    dch[mt][:, e : e + 1],
                )

    def emit_p2(g, w):
        P2C = P2C_LAST if g == n_groups - 1 else P2C_MID
        NW = n // P2C
        mts = mts_of(g)
        if p2_own_slots:
            p2 = psum.tile([128, P2C], F32, tag="p2", name="p2", bufs=2)
        else:
            p2 = psum.tile([128, P2C], F32, tag="ps", name="p2")
        for s in range(P2C // mm_chunk):
            lo = w * P2C + s * mm_chunk
            for j, mt in enumerate(mts):
                nc.tensor.matmul(
                    p2[:, s * mm_chunk : (s + 1) * mm_chunk],
                    lhsT=vb[:, mt, :],
                    rhs=expT[mt][:, lo : lo + mm_chunk],
                    start=(j == 0),
                    stop=(j == len(mts) - 1),
                )
        acc_sl = outacc[:, w * P2C : (w + 1) * P2C]
        if g == 0:
            nc.vector.tensor_copy(acc_sl, p2)
        elif g < n_groups - 1:
            nc.vector.tensor_add(acc_sl, acc_sl, p2)
        else:
            o_s = outp.tile([128, P2C], ot.dtype, tag="o_s", name="o_s")
            nc.vector.tensor_add(o_s, acc_sl, p2)
            nc.sync.dma_start(out=ot[:, w * P2C : (w + 1) * P2C], in_=o_s)
            if flags_w is not None and w == NW - 1:
                nc.vector.tensor_scalar_add(flags_w[0:1, 1:2], o_s[0:1, 0:1], 1e30)

    pending = []
    for g in range(n_groups):
        mts = mts_of(g)
        exp_per_group = groups[g] * ECH
        # Interleave the previous group's phase-2 tiles into this group's
        # phase-1 stream so the PE stays ahead of ACT without starving it.
        stride = max(1, exp_per_group // max(1, len(pending)))
        cnt = 0
        for e in range(ECH):
            for mt in mts:
                emit_exp(mt, e)
                cnt += 1
                if cnt % stride == 0 and pending:
                    emit_p2(*pending.pop(0))
        while pending:
            emit_p2(*pending.pop(0))
        for mt in mts:
            nc.vector.reduce_sum(
                denom[:, mt : mt + 1], dch[mt][:, :], axis=mybir.AxisListType.X
            )
            nc.vector.reciprocal(recip[:, mt : mt + 1], denom[:, mt : mt + 1])
            nc.vector.tensor_scalar_mul(
                vb[:, mt, :], v_s[:, mt, :], recip[:, mt : mt + 1]
            )
        if flags_w is not None and g == n_groups - 1:
            nc.vector.tensor_scalar_add(
                flags_w[0:1, 0:1], vb[0:1, MT - 1, 0:1], 1e30
            )
        nw_g = n // (P2C_LAST if g == n_groups - 1 else P2C_MID)
        pending = [(g, w) for w in range(nw_g)]
    for item in pending:
        emit_p2(*item)


def run_body(
    nc, tc, psum, outp, qt_s, kt_s, v_s, expT, dch, denom, recip, vb,
    ot, MT, ECH, MM_PER_E, NCH, mm_chunk, exp_chunk, flags_w=None, tsscr=None,
    rowsum="act",
):
    d = vb.shape[-1]
    # ---- Phase 1: scoresT = K_sh @ Q^T, exp, row-sums ----
    for mt in range(MT):
        k_col = kt_s[:, mt * 128 : (mt + 1) * 128]
        for e in range(ECH):
            ps = psum.tile([128, exp_chunk], F32, tag="ps", name="ps")
            for j in range(MM_PER_E):
                c0 = e * exp_chunk + j * mm_chunk
                nc.tensor.matmul(
                    ps[:, j * mm_chunk : (j + 1) * mm_chunk],
                    lhsT=k_col,
                    rhs=qt_s[:, c0 : c0 + mm_chunk],
                    start=True,
                    stop=True,
                )
            if rowsum == "act":
                nc.scalar.activation(
                    out=expT[mt][:, e * exp_chunk : (e + 1) * exp_chunk],
                    in_=ps,
                    func=mybir.ActivationFunctionType.Exp,
                    accum_out=dch[mt][:, e : e + 1],
                )
            else:
                nc.scalar.activation(
                    out=expT[mt][:, e * exp_chunk : (e + 1) * exp_chunk],
                    in_=ps,
                    func=mybir.ActivationFunctionType.Exp,
                )
                _exp_rowsum(
                    nc, tsscr,
                    expT[mt][:, e * exp_chunk : (e + 1) * exp_chunk],
                    dch[mt][:, e : e + 1],
                )
        nc.vector.reduce_sum(
            denom[:, mt : mt + 1], dch[mt][:, :], axis=mybir.AxisListType.X
        )
        nc.vector.reciprocal(recip[:, mt : mt + 1], denom[:, mt : mt + 1])
        nc.vector.tensor_scalar_mul(
            vb[:, mt, :], v_s[:, mt, :], recip[:, mt : mt + 1]
        )

    if flags_w is not None:
        # Flag A: phase 1 done.  Reads the final vb tile so it is ordered
        # after the last phase-1 DVE op; +1e30 makes the flip detectable.
        nc.vector.tensor_scalar_add(flags_w[0:1, 0:1], vb[0:1, MT - 1, 0:1], 1e30)

    # ---- Phase 2: outT = V'^T @ expT, accumulated over m-tiles ----
    for c in range(NCH):
        ps2 = psum.tile([128, mm_chunk], F32, tag="ps", name="ps2")
        for mt in range(MT):
            nc.tensor.matmul(
                ps2,
                lhsT=vb[:, mt, :],
                rhs=expT[mt][:, c * mm_chunk : (c + 1) * mm_chunk],
                start=(mt == 0),
                stop=(mt == MT - 1),
            )
        o_s = outp.tile([128, mm_chunk], F32)
        nc.vector.tensor_copy(o_s, ps2)
        nc.sync.dma_start(out=ot[:, c * mm_chunk : (c + 1) * mm_chunk], in_=o_s)
        if flags_w is not None and c == NCH - 1:
            # Flag B: last phase-2 PSUM->SBUF copy done (output DMAs excluded).
            nc.vector.tensor_scalar_add(flags_w[0:1, 1:2], o_s[0:1, 0:1], 1e30)


# ---------------------------------------------------------------------------
# overlap2: chunk-level ACT/DVE exp split + 3-phase p2 accumulation.
#
# exp work is split between ScalarE (native Exp, ~1.0us per 1024-chunk) and
# VectorE via a Schraudolph bit-trick (~1.5us per 1024-chunk):
#     bf16_bits(exp(s)) ~ int16(round(128*log2(e)*s + 128*(127-c)))
# i.e. one tensor_scalar (f32 PSUM -> int16 view of the bf16 expT tile)
# followed by a 4x-mode bf16 row-sum.  c = 0.044 minimises the max relative
# error (~3.3%); applied to ~1/3 of the score mass it costs ~1e-3 of final
# relative error (measured 3.7e-3 at 20/64 chunks vs 2.6e-3 all-ACT).
#
# p2 drains: group 0's p2 windows are COPIED psum->outacc by ScalarE (which
# has slack), intermediate groups are ADDED by DVE, the last group's adds
# produce o_s directly.  A small last group keeps the serial tail short.
# ---------------------------------------------------------------------------

EXP_C = 0.044
EXPA = 128.0 * float(np.log2(np.e))
EXPB = 128.0 * (127.0 - EXP_C)


def _dve_spread(n_chunks, n_dve):
    """Evenly mark n_dve of n_chunks emission slots as DVE-assigned."""
    marks = []
    for i in range(n_chunks):
        marks.append(((i + 1) * n_dve) // n_chunks > (i * n_dve) // n_chunks)
    return marks


def run_body_overlap2(
    nc, psum, outp, qt_s, kt_s, v_s, expT, dch, denom, recip, vb, outacc,
    ot, MT, ECH, MM_PER_E, mm_chunk, exp_chunk, groups, dve_per_group,
    flags_w=None, tsscr=None, p2_chunk=None,
):
    n = qt_s.shape[-1]
    n_groups = len(groups)
    starts = [sum(groups[:i]) for i in range(n_groups)]
    P2C = p2_chunk or exp_chunk
    NW = n // P2C

    def emit_exp(mt, e, use_dve):
        k_col = kt_s[:, mt * 128 : (mt + 1) * 128]
        ps = psum.tile([128, exp_chunk], F32, tag="ps", name="ps")
        for j in range(MM_PER_E):
            c0 = e * exp_chunk + j * mm_chunk
            nc.tensor.matmul(
                ps[:, j * mm_chunk : (j + 1) * mm_chunk],
                lhsT=k_col,
                rhs=qt_s[:, c0 : c0 + mm_chunk],
                start=True,
                stop=True,
            )
        sl = expT[mt][:, e * exp_chunk : (e + 1) * exp_chunk]
        if use_dve:
            nc.vector.tensor_scalar(
                out=sl.bitcast(mybir.dt.int16),
                in0=ps,
                scalar1=EXPA,
                scalar2=EXPB,
                op0=mybir.AluOpType.mult,
                op1=mybir.AluOpType.add,
            )
            _exp_rowsum(nc, tsscr, sl, dch[mt][:, e : e + 1])
        else:
            nc.scalar.activation(
                out=sl,
                in_=ps,
                func=mybir.ActivationFunctionType.Exp,
                accum_out=dch[mt][:, e : e + 1],
            )

    def emit_p2(g, w):
        mts = list(range(starts[g], starts[g] + groups[g]))
        p2 = psum.tile([128, P2C], F32, tag="ps", name="p2")
        for s in range(P2C // mm_chunk):
            lo = w * P2C + s * mm_chunk
            for j, mt in enumerate(mts):
                nc.tensor.matmul(
                    p2[:, s * mm_chunk : (s + 1) * mm_chunk],
                    lhsT=vb[:, mt, :],
                    rhs=expT[mt][:, lo : lo + mm_chunk],
                    start=(j == 0),
                    stop=(j == len(mts) - 1),
                )
        acc_sl = outacc[:, w * P2C : (w + 1) * P2C]
        if g == 0:
            nc.scalar.activation(
                out=acc_sl, in_=p2, func=mybir.ActivationFunctionType.Copy
            )
        elif g < n_groups - 1:
            nc.vector.tensor_add(acc_sl, acc_sl, p2)
        else:
            o_s = outp.tile([128, P2C], ot.dtype, tag="o_s", name="o_s")
            nc.vector.tensor_add(o_s, acc_sl, p2)
            nc.sync.dma_start(out=ot[:, w * P2C : (w + 1) * P2C], in_=o_s)
            if flags_w is not None and w == NW - 1:
                nc.vector.tensor_scalar_add(flags_w[0:1, 1:2], o_s[0:1, 0:1], 1e30)

    pending = []
    for g in range(n_groups):
        mts = list(range(starts[g], starts[g] + groups[g]))
        emits = groups[g] * ECH
        dve_marks = _dve_spread(emits, dve_per_group[g])
        stride = max(1, emits // max(1, len(pending)))
        cnt = 0
        for e in range(ECH):
            for mt in mts:
                emit_exp(mt, e, dve_marks[cnt])
                cnt += 1
                if cnt % stride == 0 and pending:
                    emit_p2(*pending.pop(0))
        while pending:
            emit_p2(*pending.pop(0))
        for mt in mts:
            nc.vector.reduce_sum(
                denom[:, mt : mt + 1], dch[mt][:, :], axis=mybir.AxisListType.X
            )
            nc.vector.reciprocal(recip[:, mt : mt + 1], denom[:, mt : mt + 1])
            nc.vector.tensor_scalar_mul(
                vb[:, mt, :], v_s[:, mt, :], recip[:, mt : mt + 1]
            )
        if flags_w is not None and g == n_groups - 1:
            nc.vector.tensor_scalar_add(
                flags_w[0:1, 0:1], vb[0:1, MT - 1, 0:1], 1e30
            )
        pending = [(g, w) for w in range(NW)]
    for item in pending:
        emit_p2(*item)


_CACHE = {}


BEST_CONFIG = dict(
    exp_chunk=1024, layout="overlap2", groups=(3, 4, 1),
    dve_per_group=(7, 10, 3), p2_chunk=1024, outp_bufs=4,
)


def _get_nc():
    if "nc" not in _CACHE:
        _CACHE["nc"] = build_attention_nc(**BEST_CONFIG)
    return _CACHE["nc"]


def make_in_maps(Q, K, V):
    Q = np.asarray(Q, dtype=np.float32)
    K = np.asarray(K, dtype=np.float32)
    V = np.asarray(V, dtype=np.float32)
    qt = np.ascontiguousarray(Q.T.astype(np.float16))
    in_maps = []
    for i in range(N_CORES):
        sl = slice(i * M_SH, (i + 1) * M_SH)
        in_maps.append(
            {
                "qt": qt,
                "kt": np.ascontiguousarray(K[sl].T.astype(np.float16)),
                "v": np.ascontiguousarray(V[sl]),
            }
        )
    return in_maps


def combine_results(per_core_ot):
    acc = np.zeros((D, N), dtype=np.float64)
    for o in per_core_ot:
        acc += o.astype(np.float64)
    return np.ascontiguousarray(acc.T).astype(np.float32)


def kernel(Q, K, V):
    in_maps = make_in_maps(Q, K, V)
    res = run_bass_kernel_spmd(_get_nc(), in_maps, core_ids=list(range(N_CORES)))
    return combine_results([r["ot"] for r in res.results])



# revision 19
# speedup vs baseline: 1.0467x; 1.0467x over previous
# Self-contained Trainium2 Bass kernel for:
#   scores = Q @ K.T            [N, M]
#   attn   = softmax(scores, axis=0)   (over queries, per key column)
#   out    = attn @ V           [N, D]
# with N = M = 8192, D = 128, float32 I/O.
#
# Sharding: K/V rows (the M axis) are split across the 8 NeuronCores.
# The softmax axis (N) stays fully local to each core, so no collectives
# are needed: each core produces a partial out^T = sum over its M-shard,
# and the host sums the 8 partials.
#
# Device algorithm (per core, M_SH = 1024):
#   scoresT = K_sh @ Q^T        [M_SH, N]   (PE, fp16 inputs, f32 PSUM)
#   expT    = exp(scoresT)      bf16, via ScalarE directly from PSUM,
#                               with fused accum_out row-sums -> denom[m]
#   V'      = V / denom[:,None] bf16 (fold softmax normalizer into V)
#   outT    = V'^T @ expT       [D, N]      (PE, bf16, accumulated in PSUM)
#
# No max-subtraction is needed: scores ~ N(0, 128), |s| < ~70, and
# exp(70) ~ 2.5e30 fits fp32/bf16 range comfortably.
#
# Layouts: the contraction dim of phase 1 is D=128, which must sit on the
# SBUF partition axis for the PE; the host passes Q^T and K_sh^T so every
# DMA is a contiguous load and the device never transposes anything.

from contextlib import ExitStack

import numpy as np

import concourse.bass as bass
import concourse.mybir as mybir
import concourse.tile as tile
from concourse import bacc
from concourse.bass_utils import run_bass_kernel_spmd

N, M, D = 8192, 8192, 128
N_CORES = 8
M_SH = M // N_CORES  # 1024

F32 = mybir.dt.float32
F16 = mybir.dt.float16
BF16 = mybir.dt.bfloat16


def build_attention_nc(
    n=N, m_sh=M_SH, d=D, mm_chunk=512, exp_chunk=2048, reps=1,
    timer_k=0, timer_p=2400, layout="serial", group_mts=2, psum_bufs=None,
    rowsum="act", p2_chunk=None, groups=None, p2_own_slots=False,
    outp_bufs=3, out_f16=False, dve_per_group=None, sep_tags=False,
    dve_rowsums=0,
):
    """Build the per-core Bass program.

    mm_chunk: free-dim (n) size of each phase-1/phase-2 matmul (<=512, one
              f32 PSUM bank per matmul).
    exp_chunk: free-dim size of each ScalarE exp op; one PSUM tile of
               exp_chunk/mm_chunk banks is filled by that many matmuls and
               consumed by a single activation instruction.
    timer_k: if >0, add an on-device timing sampler: the (otherwise idle)
             GpSimd engine snapshots four SBUF flag cells every ~timer_p
             cycles into a [4, timer_k] "tsamp" output (cells: group-0 exp
             done, group-1 exp done, phase-1 end, last phase-2 copy).  The
             flag cells are aliased SBUF tensors (alloc_sbuf_tensor_at) so
             the sampler's reads are invisible to Tile's dependency tracker
             and genuinely race with the compute - which is the whole
             point.  Timing-only variant; the graded kernel() path uses
             timer_k=0.  NOTE dev_cal.py's calibration loop must match this
             sampler's per-iteration body exactly.
    """
    assert d == 128
    assert m_sh % 128 == 0 and n % exp_chunk == 0 and exp_chunk % mm_chunk == 0
    MT = m_sh // 128           # m-tiles of 128 partitions
    ECH = n // exp_chunk       # exp chunks per m-tile
    MM_PER_E = exp_chunk // mm_chunk
    NCH = n // mm_chunk        # phase-2 output chunks

    nc = bacc.Bacc()
    qt = nc.dram_tensor("qt", [d, n], F16, kind="ExternalInput")
    kt = nc.dram_tensor("kt", [d, m_sh], F16, kind="ExternalInput")
    v = nc.dram_tensor("v", [m_sh, d], F32, kind="ExternalInput")
    ot = nc.dram_tensor("ot", [d, n], F16 if out_f16 else F32, kind="ExternalOutput")

    flags_w = flags_r = tsamp = None
    NF = 4  # flag cells: 0=g0 exp done, 1=g1 exp done, 2=phase-1 end, 3=total
    U32 = mybir.dt.int32
    if timer_k:
        tsamp = nc.dram_tensor("tsamp", [NF, timer_k], U32, kind="ExternalOutput")
        # Two views of the same SBUF bytes: the compute side writes flags_w
        # (f32), the sampler reads the raw bits through flags_r (i32;
        # reg_load/save require int dtypes).  flags_w is bump-allocated
        # normally (so nothing else lands there) and flags_r aliases its
        # offset via alloc_sbuf_tensor_at.
        flags_w = nc.alloc_sbuf_tensor("flags_w", [1, NF], F32).ap()
        flag_addr = next(
            a.memorylocations[0].addr
            for a in nc.m.functions[0].allocations
            if getattr(a, "memorylocations", None)
            and a.memorylocations[0].name.startswith("flags_w")
        )
        flags_r = nc.alloc_sbuf_tensor_at(
            "flags_r", [1, NF], U32, offset=flag_addr
        ).ap()

    with tile.TileContext(nc) as tc, ExitStack() as ctx:
        singles = ctx.enter_context(tc.tile_pool(name="singles", bufs=1))
        # One PSUM pool; phase-1 exp tiles and phase-2 accumulators share the
        # same tag, together filling all 8 banks.
        if psum_bufs is None:
            psum_bufs = 4 if layout in ("overlap", "overlap2") else 2
        psum = ctx.enter_context(
            tc.tile_pool(name="psum", bufs=psum_bufs, space="PSUM")
        )
        outp = ctx.enter_context(tc.tile_pool(name="outp", bufs=outp_bufs))

        # kt first (small, needed by the very first matmul), then qt in
        # chunks so matmul 0 isn't gated on the full 2 MB load, v last.
        kt_s = singles.tile([d, m_sh], F16)
        # Column-chunked so the first m-tile's weights (32 KB) land fast;
        # the first real matmul is gated only on chunk 0 + qt chunk 0.
        for i in range(m_sh // 256):
            nc.sync.dma_start(
                out=kt_s[:, i * 256 : (i + 1) * 256],
                in_=kt[:, i * 256 : (i + 1) * 256],
            )
        qt_s = singles.tile([d, n], F16)
        n_ld = min(max(exp_chunk, n // 8), n // 16)
        for i in range(n // n_ld):
            nc.sync.dma_start(
                out=qt_s[:, i * n_ld : (i + 1) * n_ld],
                in_=qt[:, i * n_ld : (i + 1) * n_ld],
            )
        v_s = singles.tile([128, MT, d], F32)
        nc.sync.dma_start(out=v_s, in_=v.rearrange("(t p) d -> p t d", p=128))
        # Warm the ScalarE exp table during the input-DMA window so the
        # ~2.7us ACT_TABLE_LOAD is off the critical path of the first real
        # exp op.
        actwarm = singles.tile([1, 1], F32, name="actwarm")
        nc.vector.memset(actwarm, 0.0)
        actwarm2 = singles.tile([1, 1], F32, name="actwarm2")
        nc.scalar.activation(
            out=actwarm2, in_=actwarm, func=mybir.ActivationFunctionType.Exp
        )
        # First-touch v_s on DVE: the TS (tensor_scalar) instruction format
        # has a single HW sync-wait slot, so the real consumer below must not
        # be the one that waits on this DMA.
        v_touch = singles.tile([128, 1], F32)
        nc.vector.tensor_copy(v_touch, v_s[:, 0, 0:1])

        expT = [
            singles.tile([128, n], BF16, tag=f"expT{mt}", name=f"expT{mt}")
            for mt in range(MT)
        ]
        dch = [
            singles.tile([128, ECH], F32, tag=f"dch{mt}", name=f"dch{mt}")
            for mt in range(MT)
        ]
        denom = singles.tile([128, MT], F32)
        recip = singles.tile([128, MT], F32)
        vb = singles.tile([128, MT, d], BF16)
        outacc = (
            singles.tile([128, n], F16, name="outacc")
            if layout in ("overlap", "overlap2")
            else None
        )
        # Garbage output for the DVE tensor_scalar that computes the row
        # sums (accum_out) at 4x off the bf16 expT chunks; rewritten every
        # call, same engine so pure program-order, no sync cost.
        tsscr = singles.tile([128, exp_chunk], BF16, name="tsscr")

        if timer_k:
            gp = nc.gpsimd
            gp.memset(flags_r, 0)
            samp = [singles.tile([1, timer_k], U32, name=f"samp{f}") for f in range(NF)]
            # Pacing: a Pool-engine memset of timer_p elements (~timer_p
            # cycles @1.2GHz + Q7 launch overhead).
            pace = singles.tile([1, timer_p], U32)
            regs = [gp.alloc_register(f"r{f}") for f in range(NF)]
            for i in range(timer_k):
                gp.memset(pace, 0)
                for f in range(NF):
                    gp.reg_load(regs[f], flags_r[0:1, f : f + 1])
                    gp.reg_save(samp[f][0:1, i : i + 1], regs[f])
            for f in range(NF):
                gp.dma_start(out=tsamp[f : f + 1, :], in_=samp[f])

        # reps>1 repeats the whole compute body inside one NEFF; used only by
        # the timing harness (per-dispatch overhead cancels in the delta).
        for _rep in range(reps):
            if layout == "overlap2":
                run_body_overlap2(
                    nc, psum, outp, qt_s, kt_s, v_s, expT, dch, denom, recip,
                    vb, outacc, ot, MT, ECH, MM_PER_E, mm_chunk, exp_chunk,
                    list(groups), list(dve_per_group), flags_w=flags_w,
                    tsscr=tsscr, p2_chunk=p2_chunk, sep_tags=sep_tags,
                )
            elif layout == "overlap":
                run_body_overlap(
                    nc, psum, outp, qt_s, kt_s, v_s, expT, dch, denom, recip,
                    vb, outacc, ot, MT, ECH, MM_PER_E, mm_chunk, exp_chunk,
                    group_mts, flags_w=flags_w, tsscr=tsscr, rowsum=rowsum,
                    p2_chunk=p2_chunk, groups=groups, p2_own_slots=p2_own_slots,
                    dve_rowsums=dve_rowsums,
                )
            else:
                run_body(
                    nc, tc, psum, outp, qt_s, kt_s, v_s, expT, dch, denom,
                    recip, vb, ot, MT, ECH, MM_PER_E, NCH, mm_chunk, exp_chunk,
                    flags_w=flags_w, tsscr=tsscr, rowsum=rowsum,
                )

    nc.compile()
    return nc


def _exp_rowsum(nc, tsscr, expT_slice, dch_slice):
    # Row-sum of a bf16 expT chunk on the DVE at 4x (all-SBUF, 2-byte
    # operands; the f32 accum_out scalar is exempt).  ~0.26 ns/elem vs
    # 187 ns of serial ACT time for activation(accum_out=...).
    nc.vector.tensor_scalar(
        out=tsscr[:, : expT_slice.shape[-1]],
        in0=expT_slice,
        scalar1=1.0,
        scalar2=None,
        op0=mybir.AluOpType.mult,
        op1=mybir.AluOpType.add,
        accum_out=dch_slice,
    )


def run_body_overlap(
    nc, psum, outp, qt_s, kt_s, v_s, expT, dch, denom, recip, vb, outacc,
    ot, MT, ECH, MM_PER_E, mm_chunk, exp_chunk, group_mts, flags_w=None,
    tsscr=None, rowsum="act", p2_chunk=None, groups=None, p2_own_slots=False,
    dve_rowsums=0,
):
    """Group the m-tiles; after each group's phase 1, its phase-2 partial
    (outT contribution) is emitted interleaved into the NEXT group's
    phase-1 stream, accumulating into fp16 outacc.  Only the last group's
    phase-2 remains as a serial tail (~1/n_groups of the old 28us)."""
    d = vb.shape[-1]
    n = qt_s.shape[-1]
    if groups is None:
        groups = [group_mts] * (MT // group_mts)
    assert sum(groups) == MT
    n_groups = len(groups)
    starts = [sum(groups[:i]) for i in range(n_groups)]

    def mts_of(g):
        return list(range(starts[g], starts[g] + groups[g]))
    # Interleaved groups use narrow p2 tiles (less slot-hold disruption of
    # the ACT exp feed); the final tail group uses wide ones (fewer drain
    # ops on the critical tail).
    P2C_MID = p2_chunk or exp_chunk
    # With dedicated p2 slots (2 x 1-bank), every p2 tile must fit 1 bank.
    P2C_LAST = P2C_MID if p2_own_slots else exp_chunk
    total_chunks = MT * ECH
    rs_marks = _dve_spread(total_chunks, dve_rowsums)
    rs_state = {"i": 0}

    def emit_exp(mt, e):
        if dve_rowsums:
            use_dve_rs = rs_marks[rs_state["i"]]
            rs_state["i"] += 1
            k_col = kt_s[:, mt * 128 : (mt + 1) * 128]
            ps = psum.tile([128, exp_chunk], F32, tag="ps", name="ps")
            for j in range(MM_PER_E):
                c0 = e * exp_chunk + j * mm_chunk
                nc.tensor.matmul(
                    ps[:, j * mm_chunk : (j + 1) * mm_chunk],
                    lhsT=k_col,
                    rhs=qt_s[:, c0 : c0 + mm_chunk],
                    start=True,
                    stop=True,
                )
            sl = expT[mt][:, e * exp_chunk : (e + 1) * exp_chunk]
            if use_dve_rs:
                nc.scalar.activation(
                    out=sl, in_=ps, func=mybir.ActivationFunctionType.Exp
                )
                _exp_rowsum(nc, tsscr, sl, dch[mt][:, e : e + 1])
            else:
                nc.scalar.activation(
                    out=sl,
                    in_=ps,
                    func=mybir.ActivationFunctionType.Exp,
                    accum_out=dch[mt][:, e : e + 1],
                )
            return
        _emit_exp_orig(mt, e)

    def _emit_exp_orig(mt, e):
        k_col = kt_s[:, mt * 128 : (mt + 1) * 128]
        ps = psum.tile([128, exp_chunk], F32, tag="ps", name="ps")
        for j in range(MM_PER_E):
            c0 = e * exp_chunk + j * mm_chunk
            nc.tensor.matmul(
                ps[:, j * mm_chunk : (j + 1) * mm_chunk],
                lhsT=k_col,
                rhs=qt_s[:, c0 : c0 + mm_chunk],
                start=True,
                stop=True,
            )
        if rowsum == "act":
            nc.scalar.activation(
                out=expT[mt][:, e * exp_chunk : (e + 1) * exp_chunk],
                in_=ps,
                func=mybir.ActivationFunctionType.Exp,
                accum_out=dch[mt][:, e : e + 1],
            )
        else:
            if rowsum == "act":
                nc.scalar.activation(
                    out=expT[mt][:, e * exp_chunk : (e + 1) * exp_chunk],
                    in_=ps,
                    func=mybir.ActivationFunctionType.Exp,
                    accum_out=dch[mt][:, e : e + 1],
                )
            else:
                nc.scalar.activation(
                    out=expT[mt][:, e * exp_chunk : (e + 1) * exp_chunk],
                    in_=ps,
                    func=mybir.ActivationFunctionType.Exp,
                )
                _exp_rowsum(
                    nc, tsscr,
                    expT[mt][:, e * exp_chunk : (e + 1) * exp_chunk],
                    dch[mt][:, e : e + 1],
                )

    def emit_p2(g, w):
        P2C = P2C_LAST if g == n_groups - 1 else P2C_MID
        NW = n // P2C
        mts = mts_of(g)
        if p2_own_slots:
            p2 = psum.tile([128, P2C], F32, tag="p2", name="p2", bufs=2)
        else:
            p2 = psum.tile([128, P2C], F32, tag="ps", name="p2")
        for s in range(P2C // mm_chunk):
            lo = w * P2C + s * mm_chunk
            for j, mt in enumerate(mts):
                nc.tensor.matmul(
                    p2[:, s * mm_chunk : (s + 1) * mm_chunk],
                    lhsT=vb[:, mt, :],
                    rhs=expT[mt][:, lo : lo + mm_chunk],
                    start=(j == 0),
                    stop=(j == len(mts) - 1),
                )
        acc_sl = outacc[:, w * P2C : (w + 1) * P2C]
        if g == 0:
            nc.vector.tensor_copy(acc_sl, p2)
        elif g < n_groups - 1:
            nc.vector.tensor_add(acc_sl, acc_sl, p2)
        else:
            o_s = outp.tile([128, P2C], ot.dtype, tag="o_s", name="o_s")
            nc.vector.tensor_add(o_s, acc_sl, p2)
            nc.sync.dma_start(out=ot[:, w * P2C : (w + 1) * P2C], in_=o_s)
            if flags_w is not None and w == NW - 1:
                nc.vector.tensor_scalar_add(flags_w[0:1, 3:4], o_s[0:1, 0:1], 1e30)

    pending = []
    for g in range(n_groups):
        mts = mts_of(g)
        exp_per_group = groups[g] * ECH
        # Interleave the previous group's phase-2 tiles into this group's
        # phase-1 stream so the PE stays ahead of ACT without starving it.
        stride = max(1, exp_per_group // max(1, len(pending)))
        cnt = 0
        for e in range(ECH):
            for mt in mts:
                emit_exp(mt, e)
                cnt += 1
                if cnt % stride == 0 and pending:
                    emit_p2(*pending.pop(0))
        while pending:
            emit_p2(*pending.pop(0))
        for mt in mts:
            nc.vector.reduce_sum(
                denom[:, mt : mt + 1], dch[mt][:, :], axis=mybir.AxisListType.X
            )
            nc.vector.reciprocal(recip[:, mt : mt + 1], denom[:, mt : mt + 1])
            nc.vector.tensor_scalar_mul(
                vb[:, mt, :], v_s[:, mt, :], recip[:, mt : mt + 1]
            )
        if flags_w is not None and g == n_groups - 1:
            nc.vector.tensor_scalar_add(
                flags_w[0:1, 2:3], vb[0:1, MT - 1, 0:1], 1e30
            )
        nw_g = n // (P2C_LAST if g == n_groups - 1 else P2C_MID)
        pending = [(g, w) for w in range(nw_g)]
    for item in pending:
        emit_p2(*item)


def run_body(
    nc, tc, psum, outp, qt_s, kt_s, v_s, expT, dch, denom, recip, vb,
    ot, MT, ECH, MM_PER_E, NCH, mm_chunk, exp_chunk, flags_w=None, tsscr=None,
    rowsum="act",
):
    d = vb.shape[-1]
    # ---- Phase 1: scoresT = K_sh @ Q^T, exp, row-sums ----
    for mt in range(MT):
        k_col = kt_s[:, mt * 128 : (mt + 1) * 128]
        for e in range(ECH):
            ps = psum.tile([128, exp_chunk], F32, tag="ps", name="ps")
            for j in range(MM_PER_E):
                c0 = e * exp_chunk + j * mm_chunk
                nc.tensor.matmul(
                    ps[:, j * mm_chunk : (j + 1) * mm_chunk],
                    lhsT=k_col,
                    rhs=qt_s[:, c0 : c0 + mm_chunk],
                    start=True,
                    stop=True,
                )
            if rowsum == "act":
                nc.scalar.activation(
                    out=expT[mt][:, e * exp_chunk : (e + 1) * exp_chunk],
                    in_=ps,
                    func=mybir.ActivationFunctionType.Exp,
                    accum_out=dch[mt][:, e : e + 1],
                )
            else:
                nc.scalar.activation(
                    out=expT[mt][:, e * exp_chunk : (e + 1) * exp_chunk],
                    in_=ps,
                    func=mybir.ActivationFunctionType.Exp,
                )
                _exp_rowsum(
                    nc, tsscr,
                    expT[mt][:, e * exp_chunk : (e + 1) * exp_chunk],
                    dch[mt][:, e : e + 1],
                )
        nc.vector.reduce_sum(
            denom[:, mt : mt + 1], dch[mt][:, :], axis=mybir.AxisListType.X
        )
        nc.vector.reciprocal(recip[:, mt : mt + 1], denom[:, mt : mt + 1])
        nc.vector.tensor_scalar_mul(
            vb[:, mt, :], v_s[:, mt, :], recip[:, mt : mt + 1]
        )

    if flags_w is not None:
        # Flag A: phase 1 done.  Reads the final vb tile so it is ordered
        # after the last phase-1 DVE op; +1e30 makes the flip detectable.
        nc.vector.tensor_scalar_add(flags_w[0:1, 2:3], vb[0:1, MT - 1, 0:1], 1e30)

    # ---- Phase 2: outT = V'^T @ expT, accumulated over m-tiles ----
    for c in range(NCH):
        ps2 = psum.tile([128, mm_chunk], F32, tag="ps", name="ps2")
        for mt in range(MT):
            nc.tensor.matmul(
                ps2,
                lhsT=vb[:, mt, :],
                rhs=expT[mt][:, c * mm_chunk : (c + 1) * mm_chunk],
                start=(mt == 0),
                stop=(mt == MT - 1),
            )
        o_s = outp.tile([128, mm_chunk], F32)
        nc.vector.tensor_copy(o_s, ps2)
        nc.sync.dma_start(out=ot[:, c * mm_chunk : (c + 1) * mm_chunk], in_=o_s)
        if flags_w is not None and c == NCH - 1:
            # Flag B: last phase-2 PSUM->SBUF copy done (output DMAs excluded).
            nc.vector.tensor_scalar_add(flags_w[0:1, 3:4], o_s[0:1, 0:1], 1e30)


# ---------------------------------------------------------------------------
# overlap2: chunk-level ACT/DVE exp split + 3-phase p2 accumulation.
#
# exp work is split between ScalarE (native Exp, ~1.0us per 1024-chunk) and
# VectorE via a Schraudolph bit-trick (~1.5us per 1024-chunk):
#     bf16_bits(exp(s)) ~ int16(round(128*log2(e)*s + 128*(127-c)))
# i.e. one tensor_scalar (f32 PSUM -> int16 view of the bf16 expT tile)
# followed by a 4x-mode bf16 row-sum.  c = 0.044 minimises the max relative
# error (~3.3%); applied to ~1/3 of the score mass it costs ~1e-3 of final
# relative error (measured 3.7e-3 at 20/64 chunks vs 2.6e-3 all-ACT).
#
# p2 drains: group 0's p2 windows are COPIED psum->outacc by ScalarE (which
# has slack), intermediate groups are ADDED by DVE, the last group's adds
# produce o_s directly.  A small last group keeps the serial tail short.
# ---------------------------------------------------------------------------

EXP_C = 0.044
EXPA = 128.0 * float(np.log2(np.e))
EXPB = 128.0 * (127.0 - EXP_C)


def _dve_spread(n_chunks, n_dve):
    """Evenly mark n_dve of n_chunks emission slots as DVE-assigned."""
    marks = []
    for i in range(n_chunks):
        marks.append(((i + 1) * n_dve) // n_chunks > (i * n_dve) // n_chunks)
    return marks


def run_body_overlap2(
    nc, psum, outp, qt_s, kt_s, v_s, expT, dch, denom, recip, vb, outacc,
    ot, MT, ECH, MM_PER_E, mm_chunk, exp_chunk, groups, dve_per_group,
    flags_w=None, tsscr=None, p2_chunk=None, sep_tags=False,
):
    n = qt_s.shape[-1]
    n_groups = len(groups)
    starts = [sum(groups[:i]) for i in range(n_groups)]
    P2C = p2_chunk or exp_chunk
    NW = n // P2C

    def emit_exp(mt, e, use_dve):
        k_col = kt_s[:, mt * 128 : (mt + 1) * 128]
        if sep_tags:
            ps = psum.tile(
                [128, exp_chunk], F32,
                tag="psd" if use_dve else "psa",
                name="ps", bufs=1 if use_dve else 2,
            )
        else:
            ps = psum.tile([128, exp_chunk], F32, tag="ps", name="ps")
        for j in range(MM_PER_E):
            c0 = e * exp_chunk + j * mm_chunk
            nc.tensor.matmul(
                ps[:, j * mm_chunk : (j + 1) * mm_chunk],
                lhsT=k_col,
                rhs=qt_s[:, c0 : c0 + mm_chunk],
                start=True,
                stop=True,
            )
        sl = expT[mt][:, e * exp_chunk : (e + 1) * exp_chunk]
        if use_dve:
            nc.vector.tensor_scalar(
                out=sl.bitcast(mybir.dt.int16),
                in0=ps,
                scalar1=EXPA,
                scalar2=EXPB,
                op0=mybir.AluOpType.mult,
                op1=mybir.AluOpType.add,
            )
            _exp_rowsum(nc, tsscr, sl, dch[mt][:, e : e + 1])
        else:
            nc.scalar.activation(
                out=sl,
                in_=ps,
                func=mybir.ActivationFunctionType.Exp,
                accum_out=dch[mt][:, e : e + 1],
            )

    def emit_p2(g, w):
        mts = list(range(starts[g], starts[g] + groups[g]))
        # Dedicated slot pair: p2 windows must not compete with the exp
        # backlog for PSUM slots, else their drain op head-of-line blocks
        # the consumer engine behind an allocation that can't free.
        p2 = psum.tile([128, P2C], F32, tag="p2", name="p2", bufs=2)
        for s in range(P2C // mm_chunk):
            lo = w * P2C + s * mm_chunk
            for j, mt in enumerate(mts):
                nc.tensor.matmul(
                    p2[:, s * mm_chunk : (s + 1) * mm_chunk],
                    lhsT=vb[:, mt, :],
                    rhs=expT[mt][:, lo : lo + mm_chunk],
                    start=(j == 0),
                    stop=(j == len(mts) - 1),
                )
        acc_sl = outacc[:, w * P2C : (w + 1) * P2C]
        if g == 0:
            nc.scalar.activation(
                out=acc_sl, in_=p2, func=mybir.ActivationFunctionType.Copy
            )
        elif g < n_groups - 1:
            nc.vector.tensor_add(acc_sl, acc_sl, p2)
        else:
            o_s = outp.tile([128, P2C], ot.dtype, tag="o_s", name="o_s")
            nc.vector.tensor_add(o_s, acc_sl, p2)
            nc.sync.dma_start(out=ot[:, w * P2C : (w + 1) * P2C], in_=o_s)
            if flags_w is not None and w == NW - 1:
                nc.vector.tensor_scalar_add(flags_w[0:1, 3:4], o_s[0:1, 0:1], 1e30)

    pending = []
    for g in range(n_groups):
        mts = list(range(starts[g], starts[g] + groups[g]))
        emits = groups[g] * ECH
        dve_marks = _dve_spread(emits, dve_per_group[g])
        stride = max(1, emits // max(1, len(pending)))
        cnt = 0
        for e in range(ECH):
            for mt in mts:
                emit_exp(mt, e, dve_marks[cnt])
                cnt += 1
                if cnt % stride == 0 and pending:
                    emit_p2(*pending.pop(0))
        while pending:
            emit_p2(*pending.pop(0))
        for mt in mts:
            nc.vector.reduce_sum(
                denom[:, mt : mt + 1], dch[mt][:, :], axis=mybir.AxisListType.X
            )
            nc.vector.reciprocal(recip[:, mt : mt + 1], denom[:, mt : mt + 1])
            nc.vector.tensor_scalar_mul(
                vb[:, mt, :], v_s[:, mt, :], recip[:, mt : mt + 1]
            )
        if flags_w is not None:
            cell = 2 if g == n_groups - 1 else min(g, 1)
            nc.vector.tensor_scalar_add(
                flags_w[0:1, cell : cell + 1],
                vb[0:1, mts[-1], 0:1],
                1e30,
            )
        pending = [(g, w) for w in range(NW)]
    for item in pending:
        emit_p2(*item)


_CACHE = {}


# Measured 2026-08-07 (same-process comparison, GpSimd sampler):
#   overlap/act/512       84.8us   <- fastest; ships
#   overlap+dve_rowsums   89.7us   (cross-engine rowsum sync ~+0.7us/op)
#   overlap2 (DVE exp)    86.6-104us (ACT+DVE psum-slot convoying)
# The single-ACT-consumer fused pipeline is the local optimum; exp/rowsum
# offload variants (overlap2 / dve_rowsums) are kept for reference.
BEST_CONFIG = dict(
    exp_chunk=1024, layout="overlap", group_mts=2, rowsum="act",
    p2_chunk=512, outp_bufs=5,
)


def _get_nc():
    if "nc" not in _CACHE:
        _CACHE["nc"] = build_attention_nc(**BEST_CONFIG)
    return _CACHE["nc"]


def make_in_maps(Q, K, V):
    Q = np.asarray(Q, dtype=np.float32)
    K = np.asarray(K, dtype=np.float32)
    V = np.asarray(V, dtype=np.float32)
    qt = np.ascontiguousarray(Q.T.astype(np.float16))
    in_maps = []
    for i in range(N_CORES):
        sl = slice(i * M_SH, (i + 1) * M_SH)
        in_maps.append(
            {
                "qt": qt,
                "kt": np.ascontiguousarray(K[sl].T.astype(np.float16)),
                "v": np.ascontiguousarray(V[sl]),
            }
        )
    return in_maps


def combine_results(per_core_ot):
    acc = np.zeros((D, N), dtype=np.float64)
    for o in per_core_ot:
        acc += o.astype(np.float64)
    return np.ascontiguousarray(acc.T).astype(np.float32)


def kernel(Q, K, V):
    in_maps = make_in_maps(Q, K, V)
    res = run_bass_kernel_spmd(_get_nc(), in_maps, core_ids=list(range(N_CORES)))
    return combine_results([r["ot"] for r in res.results])



# revision 25
# speedup vs baseline: 1.0520x; 1.0051x over previous
# Self-contained Trainium2 Bass kernel for:
#   scores = Q @ K.T            [N, M]
#   attn   = softmax(scores, axis=0)   (over queries, per key column)
#   out    = attn @ V           [N, D]
# with N = M = 8192, D = 128, float32 I/O.
#
# Sharding: K/V rows (the M axis) are split across the 8 NeuronCores.
# The softmax axis (N) stays fully local to each core, so no collectives
# are needed: each core produces a partial out^T = sum over its M-shard,
# and the host sums the 8 partials.
#
# Device algorithm (per core, M_SH = 1024):
#   scoresT = K_sh @ Q^T        [M_SH, N]   (PE, fp16 inputs, f32 PSUM)
#   expT    = exp(scoresT)      bf16, via ScalarE directly from PSUM,
#                               with fused accum_out row-sums -> denom[m]
#   V'      = V / denom[:,None] bf16 (fold softmax normalizer into V)
#   outT    = V'^T @ expT       [D, N]      (PE, bf16, accumulated in PSUM)
#
# No max-subtraction is needed: scores ~ N(0, 128), |s| < ~70, and
# exp(70) ~ 2.5e30 fits fp32/bf16 range comfortably.
#
# Layouts: the contraction dim of phase 1 is D=128, which must sit on the
# SBUF partition axis for the PE; the host passes Q^T and K_sh^T so every
# DMA is a contiguous load and the device never transposes anything.

from contextlib import ExitStack

import numpy as np

import concourse.bass as bass
import concourse.mybir as mybir
import concourse.tile as tile
from concourse import bacc
from concourse.bass_utils import run_bass_kernel_spmd

N, M, D = 8192, 8192, 128
N_CORES = 8
M_SH = M // N_CORES  # 1024

F32 = mybir.dt.float32
F16 = mybir.dt.float16
BF16 = mybir.dt.bfloat16


def build_attention_nc(
    n=N, m_sh=M_SH, d=D, mm_chunk=512, exp_chunk=2048, reps=1,
    timer_k=0, timer_p=2400, layout="serial", group_mts=2, psum_bufs=None,
    rowsum="act", p2_chunk=None, groups=None, p2_own_slots=False,
    outp_bufs=3, out_f16=False, dve_per_group=None, sep_tags=False,
    dve_rowsums=0, dve_exp_tiles=(),
):
    """Build the per-core Bass program.

    mm_chunk: free-dim (n) size of each phase-1/phase-2 matmul (<=512, one
              f32 PSUM bank per matmul).
    exp_chunk: free-dim size of each ScalarE exp op; one PSUM tile of
               exp_chunk/mm_chunk banks is filled by that many matmuls and
               consumed by a single activation instruction.
    timer_k: if >0, add an on-device timing sampler: the (otherwise idle)
             GpSimd engine snapshots four SBUF flag cells every ~timer_p
             cycles into a [4, timer_k] "tsamp" output (cells: group-0 exp
             done, group-1 exp done, phase-1 end, last phase-2 copy).  The
             flag cells are aliased SBUF tensors (alloc_sbuf_tensor_at) so
             the sampler's reads are invisible to Tile's dependency tracker
             and genuinely race with the compute - which is the whole
             point.  Timing-only variant; the graded kernel() path uses
             timer_k=0.  NOTE dev_cal.py's calibration loop must match this
             sampler's per-iteration body exactly.
    """
    assert d == 128
    assert m_sh % 128 == 0 and n % exp_chunk == 0 and exp_chunk % mm_chunk == 0
    MT = m_sh // 128           # m-tiles of 128 partitions
    ECH = n // exp_chunk       # exp chunks per m-tile
    MM_PER_E = exp_chunk // mm_chunk
    NCH = n // mm_chunk        # phase-2 output chunks

    nc = bacc.Bacc()
    # Inputs arrive chunk-major ([chunk, d, cols]) so every DMA is one
    # contiguous block instead of 128 strided 1 KB lines - the [d, n]
    # layout starved phase 1 (m-tile 0 streams ALL of qt in ~8.5 us).
    QTC, KTC = 512, 256
    qt = nc.dram_tensor("qt", [n // QTC, d, QTC], F16, kind="ExternalInput")
    kt = nc.dram_tensor("kt", [m_sh // KTC, d, KTC], F16, kind="ExternalInput")
    # v pre-tiled on host to [128, MT, d] so the load is one contiguous
    # 4 KB/partition DMA instead of a 512 B-line strided gather.
    v = nc.dram_tensor("v", [128, m_sh // 128, d], F32, kind="ExternalInput")
    ot = nc.dram_tensor("ot", [d, n], F16 if out_f16 else F32, kind="ExternalOutput")

    flags_w = flags_r = tsamp = None
    NF = 4  # flag cells: 0=g0 exp done, 1=g1 exp done, 2=phase-1 end, 3=total
    U32 = mybir.dt.int32
    if timer_k:
        tsamp = nc.dram_tensor("tsamp", [NF, timer_k], U32, kind="ExternalOutput")
        # Two views of the same SBUF bytes: the compute side writes flags_w
        # (f32), the sampler reads the raw bits through flags_r (i32;
        # reg_load/save require int dtypes).  flags_w is bump-allocated
        # normally (so nothing else lands there) and flags_r aliases its
        # offset via alloc_sbuf_tensor_at.
        flags_w = nc.alloc_sbuf_tensor("flags_w", [1, NF], F32).ap()
        flag_addr = next(
            a.memorylocations[0].addr
            for a in nc.m.functions[0].allocations
            if getattr(a, "memorylocations", None)
            and a.memorylocations[0].name.startswith("flags_w")
        )
        flags_r = nc.alloc_sbuf_tensor_at(
            "flags_r", [1, NF], U32, offset=flag_addr
        ).ap()

    with tile.TileContext(nc) as tc, ExitStack() as ctx:
        singles = ctx.enter_context(tc.tile_pool(name="singles", bufs=1))
        # One PSUM pool; phase-1 exp tiles and phase-2 accumulators share the
        # same tag, together filling all 8 banks.
        if psum_bufs is None:
            psum_bufs = 4 if layout in ("overlap", "overlap2") else 2
        psum = ctx.enter_context(
            tc.tile_pool(name="psum", bufs=psum_bufs, space="PSUM")
        )
        outp = ctx.enter_context(tc.tile_pool(name="outp", bufs=outp_bufs))

        # kt first (small, needed by the very first matmul), then qt in
        # chunks so matmul 0 isn't gated on the full 2 MB load, v last.
        kt_s = singles.tile([d, m_sh], F16)
        for i in range(m_sh // KTC):
            nc.sync.dma_start(
                out=kt_s[:, i * KTC : (i + 1) * KTC], in_=kt[i]
            )
        # Stripe the 2 MB qt load across four engines' DMA queues - a single
        # queue moves ~138 GB/s (measured: qt resident at 14.5 us), which
        # starved m-tile 0's full-width sweep.  The compute engines are idle
        # at kernel start, so the trigger instructions cost nothing.
        qt_s = singles.tile([d, n], F16)
        dma_engines = [nc.sync, nc.scalar]
        for i in range(n // QTC):
            dma_engines[i % len(dma_engines)].dma_start(
                out=qt_s[:, i * QTC : (i + 1) * QTC], in_=qt[i][:, :]
            )
        v_s = singles.tile([128, MT, d], F32)
        nc.scalar.dma_start(out=v_s, in_=v[:, :, :])
        # Warm the ScalarE exp table during the input-DMA window so the
        # ~2.7us ACT_TABLE_LOAD is off the critical path of the first real
        # exp op.
        actwarm = singles.tile([1, 1], F32, name="actwarm")
        nc.vector.memset(actwarm, 0.0)
        actwarm2 = singles.tile([1, 1], F32, name="actwarm2")
        nc.scalar.activation(
            out=actwarm2, in_=actwarm, func=mybir.ActivationFunctionType.Exp
        )
        # First-touch v_s on DVE: the TS (tensor_scalar) instruction format
        # has a single HW sync-wait slot, so the real consumer below must not
        # be the one that waits on this DMA.
        v_touch = singles.tile([128, 1], F32)
        nc.vector.tensor_copy(v_touch, v_s[:, 0, 0:1])

        expT = [
            singles.tile([128, n], BF16, tag=f"expT{mt}", name=f"expT{mt}")
            for mt in range(MT)
        ]
        dch = [
            singles.tile([128, ECH], F32, tag=f"dch{mt}", name=f"dch{mt}")
            for mt in range(MT)
        ]
        denom = singles.tile([128, MT], F32)
        recip = singles.tile([128, MT], F32)
        vb = singles.tile([128, MT, d], BF16)
        outacc = (
            singles.tile([128, n], F16, name="outacc")
            if layout in ("overlap", "overlap2")
            else None
        )
        # Garbage output for the DVE tensor_scalar that computes the row
        # sums (accum_out) at 4x off the bf16 expT chunks; rewritten every
        # call, same engine so pure program-order, no sync cost.
        tsscr = singles.tile([128, exp_chunk], BF16, name="tsscr")

        if timer_k:
            gp = nc.gpsimd
            gp.memset(flags_r, 0)
            samp = [singles.tile([1, timer_k], U32, name=f"samp{f}") for f in range(NF)]
            # Pacing: a Pool-engine memset of timer_p elements (~timer_p
            # cycles @1.2GHz + Q7 launch overhead).
            pace = singles.tile([1, timer_p], U32)
            regs = [gp.alloc_register(f"r{f}") for f in range(NF)]
            for i in range(timer_k):
                gp.memset(pace, 0)
                for f in range(NF):
                    gp.reg_load(regs[f], flags_r[0:1, f : f + 1])
                    gp.reg_save(samp[f][0:1, i : i + 1], regs[f])
            for f in range(NF):
                gp.dma_start(out=tsamp[f : f + 1, :], in_=samp[f])

        # reps>1 repeats the whole compute body inside one NEFF; used only by
        # the timing harness (per-dispatch overhead cancels in the delta).
        for _rep in range(reps):
            if layout == "overlap2":
                run_body_overlap2(
                    nc, psum, outp, qt_s, kt_s, v_s, expT, dch, denom, recip,
                    vb, outacc, ot, MT, ECH, MM_PER_E, mm_chunk, exp_chunk,
                    list(groups), list(dve_per_group), flags_w=flags_w,
                    tsscr=tsscr, p2_chunk=p2_chunk, sep_tags=sep_tags,
                )
            elif layout == "overlap":
                run_body_overlap(
                    nc, psum, outp, qt_s, kt_s, v_s, expT, dch, denom, recip,
                    vb, outacc, ot, MT, ECH, MM_PER_E, mm_chunk, exp_chunk,
                    group_mts, flags_w=flags_w, tsscr=tsscr, rowsum=rowsum,
                    p2_chunk=p2_chunk, groups=groups, p2_own_slots=p2_own_slots,
                    dve_rowsums=dve_rowsums, dve_exp_tiles=tuple(dve_exp_tiles),
                )
            else:
                run_body(
                    nc, tc, psum, outp, qt_s, kt_s, v_s, expT, dch, denom,
                    recip, vb, ot, MT, ECH, MM_PER_E, NCH, mm_chunk, exp_chunk,
                    flags_w=flags_w, tsscr=tsscr, rowsum=rowsum,
                )

    nc.compile()
    return nc


def _exp_rowsum(nc, tsscr, expT_slice, dch_slice):
    # Row-sum of a bf16 expT chunk on the DVE at 4x (all-SBUF, 2-byte
    # operands; the f32 accum_out scalar is exempt).  ~0.26 ns/elem vs
    # 187 ns of serial ACT time for activation(accum_out=...).
    nc.vector.tensor_scalar(
        out=tsscr[:, : expT_slice.shape[-1]],
        in0=expT_slice,
        scalar1=1.0,
        scalar2=None,
        op0=mybir.AluOpType.mult,
        op1=mybir.AluOpType.add,
        accum_out=dch_slice,
    )


def run_body_overlap(
    nc, psum, outp, qt_s, kt_s, v_s, expT, dch, denom, recip, vb, outacc,
    ot, MT, ECH, MM_PER_E, mm_chunk, exp_chunk, group_mts, flags_w=None,
    tsscr=None, rowsum="act", p2_chunk=None, groups=None, p2_own_slots=False,
    dve_rowsums=0, dve_exp_tiles=(),
):
    """Group the m-tiles; after each group's phase 1, its phase-2 partial
    (outT contribution) is emitted interleaved into the NEXT group's
    phase-1 stream, accumulating into fp16 outacc.  Only the last group's
    phase-2 remains as a serial tail (~1/n_groups of the old 28us)."""
    d = vb.shape[-1]
    n = qt_s.shape[-1]
    if groups is None:
        groups = [group_mts] * (MT // group_mts)
    assert sum(groups) == MT
    n_groups = len(groups)
    starts = [sum(groups[:i]) for i in range(n_groups)]

    def mts_of(g):
        return list(range(starts[g], starts[g] + groups[g]))

    if flags_w is not None:
        # Flag 0: last qt chunk resident (DVE op gated on its DMA).
        nc.vector.tensor_scalar_add(
            flags_w[0:1, 0:1], qt_s[0:1, qt_s.shape[-1] - 1 :], 1e30
        )
    # Interleaved groups use narrow p2 tiles (less slot-hold disruption of
    # the ACT exp feed); the final tail group uses wide ones (fewer drain
    # ops on the critical tail).
    P2C_MID = p2_chunk or exp_chunk
    # With dedicated p2 slots (2 x 1-bank) the mid-group tiles must fit one
    # bank; the last group's tiles go back to the (then idle) shared slots
    # at full width so the tail drain count stays low.
    P2C_LAST = exp_chunk
    total_chunks = MT * ECH
    rs_marks = _dve_spread(total_chunks, dve_rowsums)
    rs_state = {"i": 0}

    def emit_exp(mt, e):
        if dve_rowsums or dve_exp_tiles:
            use_dve_rs = rs_marks[rs_state["i"]] if dve_rowsums else False
            rs_state["i"] += 1
            dve_tile = mt in dve_exp_tiles
            k_col = kt_s[:, mt * 128 : (mt + 1) * 128]
            # DVE-exp tiles stream through their own 1-slot tag so the main
            # ACT exp rotation never waits on a DVE drain.
            ps = psum.tile(
                [128, exp_chunk], F32,
                tag="psd" if dve_tile else "ps",
                name="ps", bufs=1 if dve_tile else None,
            )
            for j in range(MM_PER_E):
                c0 = e * exp_chunk + j * mm_chunk
                nc.tensor.matmul(
                    ps[:, j * mm_chunk : (j + 1) * mm_chunk],
                    lhsT=k_col,
                    rhs=qt_s[:, c0 : c0 + mm_chunk],
                    start=True,
                    stop=True,
                )
            sl = expT[mt][:, e * exp_chunk : (e + 1) * exp_chunk]
            if dve_tile:
                nc.vector.tensor_scalar(
                    out=sl.bitcast(mybir.dt.int16),
                    in0=ps,
                    scalar1=EXPA,
                    scalar2=EXPB,
                    op0=mybir.AluOpType.mult,
                    op1=mybir.AluOpType.add,
                )
                _exp_rowsum(nc, tsscr, sl, dch[mt][:, e : e + 1])
            elif use_dve_rs:
                nc.scalar.activation(
                    out=sl, in_=ps, func=mybir.ActivationFunctionType.Exp
                )
                _exp_rowsum(nc, tsscr, sl, dch[mt][:, e : e + 1])
            else:
                nc.scalar.activation(
                    out=sl,
                    in_=ps,
                    func=mybir.ActivationFunctionType.Exp,
                    accum_out=dch[mt][:, e : e + 1],
                )
            return
        _emit_exp_orig(mt, e)

    def _emit_exp_orig(mt, e):
        k_col = kt_s[:, mt * 128 : (mt + 1) * 128]
        ps = psum.tile([128, exp_chunk], F32, tag="ps", name="ps")
        for j in range(MM_PER_E):
            c0 = e * exp_chunk + j * mm_chunk
            nc.tensor.matmul(
                ps[:, j * mm_chunk : (j + 1) * mm_chunk],
                lhsT=k_col,
                rhs=qt_s[:, c0 : c0 + mm_chunk],
                start=True,
                stop=True,
            )
        if rowsum == "act":
            nc.scalar.activation(
                out=expT[mt][:, e * exp_chunk : (e + 1) * exp_chunk],
                in_=ps,
                func=mybir.ActivationFunctionType.Exp,
                accum_out=dch[mt][:, e : e + 1],
            )
        else:
            if rowsum == "act":
                nc.scalar.activation(
                    out=expT[mt][:, e * exp_chunk : (e + 1) * exp_chunk],
                    in_=ps,
                    func=mybir.ActivationFunctionType.Exp,
                    accum_out=dch[mt][:, e : e + 1],
                )
            else:
                nc.scalar.activation(
                    out=expT[mt][:, e * exp_chunk : (e + 1) * exp_chunk],
                    in_=ps,
                    func=mybir.ActivationFunctionType.Exp,
                )
                _exp_rowsum(
                    nc, tsscr,
                    expT[mt][:, e * exp_chunk : (e + 1) * exp_chunk],
                    dch[mt][:, e : e + 1],
                )

    def emit_p2(g, w):
        P2C = P2C_LAST if g == n_groups - 1 else P2C_MID
        NW = n // P2C
        mts = mts_of(g)
        if p2_own_slots and g < n_groups - 1:
            p2 = psum.tile([128, P2C], F32, tag="p2", name="p2", bufs=2)
        else:
            p2 = psum.tile([128, P2C], F32, tag="ps", name="p2")
        for s in range(P2C // mm_chunk):
            lo = w * P2C + s * mm_chunk
            for j, mt in enumerate(mts):
                nc.tensor.matmul(
                    p2[:, s * mm_chunk : (s + 1) * mm_chunk],
                    lhsT=vb[:, mt, :],
                    rhs=expT[mt][:, lo : lo + mm_chunk],
                    start=(j == 0),
                    stop=(j == len(mts) - 1),
                )
        acc_sl = outacc[:, w * P2C : (w + 1) * P2C]
        if g == 0:
            nc.vector.tensor_copy(acc_sl, p2)
        elif g < n_groups - 1:
            nc.vector.tensor_add(acc_sl, acc_sl, p2)
        else:
            o_s = outp.tile([128, P2C], ot.dtype, tag="o_s", name="o_s")
            nc.vector.tensor_add(o_s, acc_sl, p2)
            nc.sync.dma_start(out=ot[:, w * P2C : (w + 1) * P2C], in_=o_s)
            if flags_w is not None and w == NW - 1:
                nc.vector.tensor_scalar_add(flags_w[0:1, 3:4], o_s[0:1, 0:1], 1e30)

    pending = []
    for g in range(n_groups):
        mts = mts_of(g)
        exp_per_group = groups[g] * ECH
        # Interleave the previous group's phase-2 tiles into this group's
        # phase-1 stream so the PE stays ahead of ACT without starving it.
        stride = max(1, exp_per_group // max(1, len(pending)))
        cnt = 0
        for e in range(ECH):
            for mt in mts:
                emit_exp(mt, e)
                cnt += 1
                if cnt % stride == 0 and pending:
                    emit_p2(*pending.pop(0))
        while pending:
            emit_p2(*pending.pop(0))
        for mt in mts:
            nc.vector.reduce_sum(
                denom[:, mt : mt + 1], dch[mt][:, :], axis=mybir.AxisListType.X
            )
            nc.vector.reciprocal(recip[:, mt : mt + 1], denom[:, mt : mt + 1])
            nc.vector.tensor_scalar_mul(
                vb[:, mt, :], v_s[:, mt, :], recip[:, mt : mt + 1]
            )
        if flags_w is not None and g == 0:
            nc.vector.tensor_scalar_add(
                flags_w[0:1, 1:2], vb[0:1, mts[-1], 0:1], 1e30
            )
        if flags_w is not None and g == n_groups - 1:
            nc.vector.tensor_scalar_add(
                flags_w[0:1, 2:3], vb[0:1, MT - 1, 0:1], 1e30
            )
        nw_g = n // (P2C_LAST if g == n_groups - 1 else P2C_MID)
        pending = [(g, w) for w in range(nw_g)]
    for item in pending:
        emit_p2(*item)


def run_body(
    nc, tc, psum, outp, qt_s, kt_s, v_s, expT, dch, denom, recip, vb,
    ot, MT, ECH, MM_PER_E, NCH, mm_chunk, exp_chunk, flags_w=None, tsscr=None,
    rowsum="act",
):
    d = vb.shape[-1]
    # ---- Phase 1: scoresT = K_sh @ Q^T, exp, row-sums ----
    for mt in range(MT):
        k_col = kt_s[:, mt * 128 : (mt + 1) * 128]
        for e in range(ECH):
            ps = psum.tile([128, exp_chunk], F32, tag="ps", name="ps")
            for j in range(MM_PER_E):
                c0 = e * exp_chunk + j * mm_chunk
                nc.tensor.matmul(
                    ps[:, j * mm_chunk : (j + 1) * mm_chunk],
                    lhsT=k_col,
                    rhs=qt_s[:, c0 : c0 + mm_chunk],
                    start=True,
                    stop=True,
                )
            if rowsum == "act":
                nc.scalar.activation(
                    out=expT[mt][:, e * exp_chunk : (e + 1) * exp_chunk],
                    in_=ps,
                    func=mybir.ActivationFunctionType.Exp,
                    accum_out=dch[mt][:, e : e + 1],
                )
            else:
                nc.scalar.activation(
                    out=expT[mt][:, e * exp_chunk : (e + 1) * exp_chunk],
                    in_=ps,
                    func=mybir.ActivationFunctionType.Exp,
                )
                _exp_rowsum(
                    nc, tsscr,
                    expT[mt][:, e * exp_chunk : (e + 1) * exp_chunk],
                    dch[mt][:, e : e + 1],
                )
        nc.vector.reduce_sum(
            denom[:, mt : mt + 1], dch[mt][:, :], axis=mybir.AxisListType.X
        )
        nc.vector.reciprocal(recip[:, mt : mt + 1], denom[:, mt : mt + 1])
        nc.vector.tensor_scalar_mul(
            vb[:, mt, :], v_s[:, mt, :], recip[:, mt : mt + 1]
        )

    if flags_w is not None:
        # Flag A: phase 1 done.  Reads the final vb tile so it is ordered
        # after the last phase-1 DVE op; +1e30 makes the flip detectable.
        nc.vector.tensor_scalar_add(flags_w[0:1, 2:3], vb[0:1, MT - 1, 0:1], 1e30)

    # ---- Phase 2: outT = V'^T @ expT, accumulated over m-tiles ----
    for c in range(NCH):
        ps2 = psum.tile([128, mm_chunk], F32, tag="ps", name="ps2")
        for mt in range(MT):
            nc.tensor.matmul(
                ps2,
                lhsT=vb[:, mt, :],
                rhs=expT[mt][:, c * mm_chunk : (c + 1) * mm_chunk],
                start=(mt == 0),
                stop=(mt == MT - 1),
            )
        o_s = outp.tile([128, mm_chunk], F32)
        nc.vector.tensor_copy(o_s, ps2)
        nc.sync.dma_start(out=ot[:, c * mm_chunk : (c + 1) * mm_chunk], in_=o_s)
        if flags_w is not None and c == NCH - 1:
            # Flag B: last phase-2 PSUM->SBUF copy done (output DMAs excluded).
            nc.vector.tensor_scalar_add(flags_w[0:1, 3:4], o_s[0:1, 0:1], 1e30)


# ---------------------------------------------------------------------------
# overlap2: chunk-level ACT/DVE exp split + 3-phase p2 accumulation.
#
# exp work is split between ScalarE (native Exp, ~1.0us per 1024-chunk) and
# VectorE via a Schraudolph bit-trick (~1.5us per 1024-chunk):
#     bf16_bits(exp(s)) ~ int16(round(128*log2(e)*s + 128*(127-c)))
# i.e. one tensor_scalar (f32 PSUM -> int16 view of the bf16 expT tile)
# followed by a 4x-mode bf16 row-sum.  c = 0.044 minimises the max relative
# error (~3.3%); applied to ~1/3 of the score mass it costs ~1e-3 of final
# relative error (measured 3.7e-3 at 20/64 chunks vs 2.6e-3 all-ACT).
#
# p2 drains: group 0's p2 windows are COPIED psum->outacc by ScalarE (which
# has slack), intermediate groups are ADDED by DVE, the last group's adds
# produce o_s directly.  A small last group keeps the serial tail short.
# ---------------------------------------------------------------------------

EXP_C = 0.044
EXPA = 128.0 * float(np.log2(np.e))
EXPB = 128.0 * (127.0 - EXP_C)


def _dve_spread(n_chunks, n_dve):
    """Evenly mark n_dve of n_chunks emission slots as DVE-assigned."""
    marks = []
    for i in range(n_chunks):
        marks.append(((i + 1) * n_dve) // n_chunks > (i * n_dve) // n_chunks)
    return marks


def run_body_overlap2(
    nc, psum, outp, qt_s, kt_s, v_s, expT, dch, denom, recip, vb, outacc,
    ot, MT, ECH, MM_PER_E, mm_chunk, exp_chunk, groups, dve_per_group,
    flags_w=None, tsscr=None, p2_chunk=None, sep_tags=False,
):
    n = qt_s.shape[-1]
    n_groups = len(groups)
    starts = [sum(groups[:i]) for i in range(n_groups)]
    P2C = p2_chunk or exp_chunk
    NW = n // P2C

    def emit_exp(mt, e, use_dve):
        k_col = kt_s[:, mt * 128 : (mt + 1) * 128]
        if sep_tags:
            ps = psum.tile(
                [128, exp_chunk], F32,
                tag="psd" if use_dve else "psa",
                name="ps", bufs=1 if use_dve else 2,
            )
        else:
            ps = psum.tile([128, exp_chunk], F32, tag="ps", name="ps")
        for j in range(MM_PER_E):
            c0 = e * exp_chunk + j * mm_chunk
            nc.tensor.matmul(
                ps[:, j * mm_chunk : (j + 1) * mm_chunk],
                lhsT=k_col,
                rhs=qt_s[:, c0 : c0 + mm_chunk],
                start=True,
                stop=True,
            )
        sl = expT[mt][:, e * exp_chunk : (e + 1) * exp_chunk]
        if use_dve:
            nc.vector.tensor_scalar(
                out=sl.bitcast(mybir.dt.int16),
                in0=ps,
                scalar1=EXPA,
                scalar2=EXPB,
                op0=mybir.AluOpType.mult,
                op1=mybir.AluOpType.add,
            )
            _exp_rowsum(nc, tsscr, sl, dch[mt][:, e : e + 1])
        else:
            nc.scalar.activation(
                out=sl,
                in_=ps,
                func=mybir.ActivationFunctionType.Exp,
                accum_out=dch[mt][:, e : e + 1],
            )

    def emit_p2(g, w):
        mts = list(range(starts[g], starts[g] + groups[g]))
        # Dedicated slot pair: p2 windows must not compete with the exp
        # backlog for PSUM slots, else their drain op head-of-line blocks
        # the consumer engine behind an allocation that can't free.
        p2 = psum.tile([128, P2C], F32, tag="p2", name="p2", bufs=2)
        for s in range(P2C // mm_chunk):
            lo = w * P2C + s * mm_chunk
            for j, mt in enumerate(mts):
                nc.tensor.matmul(
                    p2[:, s * mm_chunk : (s + 1) * mm_chunk],
                    lhsT=vb[:, mt, :],
                    rhs=expT[mt][:, lo : lo + mm_chunk],
                    start=(j == 0),
                    stop=(j == len(mts) - 1),
                )
        acc_sl = outacc[:, w * P2C : (w + 1) * P2C]
        if g == 0:
            nc.scalar.activation(
                out=acc_sl, in_=p2, func=mybir.ActivationFunctionType.Copy
            )
        elif g < n_groups - 1:
            nc.vector.tensor_add(acc_sl, acc_sl, p2)
        else:
            o_s = outp.tile([128, P2C], ot.dtype, tag="o_s", name="o_s")
            nc.vector.tensor_add(o_s, acc_sl, p2)
            nc.sync.dma_start(out=ot[:, w * P2C : (w + 1) * P2C], in_=o_s)
            if flags_w is not None and w == NW - 1:
                nc.vector.tensor_scalar_add(flags_w[0:1, 3:4], o_s[0:1, 0:1], 1e30)

    pending = []
    for g in range(n_groups):
        mts = list(range(starts[g], starts[g] + groups[g]))
        emits = groups[g] * ECH
        dve_marks = _dve_spread(emits, dve_per_group[g])
        stride = max(1, emits // max(1, len(pending)))
        cnt = 0
        for e in range(ECH):
            for mt in mts:
                emit_exp(mt, e, dve_marks[cnt])
                cnt += 1
                if cnt % stride == 0 and pending:
                    emit_p2(*pending.pop(0))
        while pending:
            emit_p2(*pending.pop(0))
        for mt in mts:
            nc.vector.reduce_sum(
                denom[:, mt : mt + 1], dch[mt][:, :], axis=mybir.AxisListType.X
            )
            nc.vector.reciprocal(recip[:, mt : mt + 1], denom[:, mt : mt + 1])
            nc.vector.tensor_scalar_mul(
                vb[:, mt, :], v_s[:, mt, :], recip[:, mt : mt + 1]
            )
        if flags_w is not None:
            cell = 2 if g == n_groups - 1 else min(g, 1)
            nc.vector.tensor_scalar_add(
                flags_w[0:1, cell : cell + 1],
                vb[0:1, mts[-1], 0:1],
                1e30,
            )
        pending = [(g, w) for w in range(NW)]
    for item in pending:
        emit_p2(*item)


_CACHE = {}


# Measured 2026-08-07 (same-process comparison, GpSimd sampler):
#   overlap/act/512       84.8us   <- fastest; ships
#   overlap+dve_rowsums   89.7us   (cross-engine rowsum sync ~+0.7us/op)
#   overlap2 (DVE exp)    86.6-104us (ACT+DVE psum-slot convoying)
# The single-ACT-consumer fused pipeline is the local optimum; exp/rowsum
# offload variants (overlap2 / dve_rowsums) are kept for reference.
BEST_CONFIG = dict(
    exp_chunk=1024, layout="overlap", group_mts=2, rowsum="act",
    p2_chunk=512, outp_bufs=5,
)


def _get_nc():
    if "nc" not in _CACHE:
        _CACHE["nc"] = build_attention_nc(**BEST_CONFIG)
    return _CACHE["nc"]


def make_in_maps(Q, K, V):
    Q = np.asarray(Q, dtype=np.float32)
    K = np.asarray(K, dtype=np.float32)
    V = np.asarray(V, dtype=np.float32)
    QTC, KTC = 512, 256
    qt = Q.T.astype(np.float16)  # [D, N]
    qt_t = np.ascontiguousarray(
        qt.reshape(D, N // QTC, QTC).transpose(1, 0, 2)
    )  # [N/QTC, D, QTC], each chunk contiguous
    in_maps = []
    for i in range(N_CORES):
        sl = slice(i * M_SH, (i + 1) * M_SH)
        kt = K[sl].T.astype(np.float16)  # [D, M_SH]
        kt_t = np.ascontiguousarray(
            kt.reshape(D, M_SH // KTC, KTC).transpose(1, 0, 2)
        )
        v_t = np.ascontiguousarray(
            V[sl].reshape(M_SH // 128, 128, D).transpose(1, 0, 2)
        )  # [128, MT, D]
        in_maps.append(
            {
                "qt": qt_t,
                "kt": kt_t,
                "v": v_t,
            }
        )
    return in_maps


def combine_results(per_core_ot):
    acc = np.zeros((D, N), dtype=np.float64)
    for o in per_core_ot:
        acc += o.astype(np.float64)
    return np.ascontiguousarray(acc.T).astype(np.float32)


def kernel(Q, K, V):
    in_maps = make_in_maps(Q, K, V)
    res = run_bass_kernel_spmd(_get_nc(), in_maps, core_ids=list(range(N_CORES)))
    return combine_results([r["ot"] for r in res.results])

